# revision 21
# baseline (speedup 1.0000x reference)
"""Trainium2 Bass kernel for a 4-layer dense transformer encoder with BatchNorm.

Model (from reference):
  B=128, S=256, D=512, L=4, V=96, H=8, FF=512, DH=64, eps=1e-3
  x = embed[sequence] + pos
  per layer: MHA -> BN(h+attn) -> FFN(relu) -> BN(h+ffn)   (BN in training mode,
  stats over (batch, seq) per feature)

Sharding: data-parallel over batch across 8 cores (16 batches / core).
BN stats are all-reduced (sum, sumsq per feature = 4KB) across cores.

Device layout: activations are kept feature-major ("transposed"):
  hT[feat, token] with feat on partitions (4 tiles of 128) and 4096 tokens free.
All six projections per layer are then natural PE matmuls
  (lhsT = W[feat_in, feat_out], rhs = hT) and BN per-feature scalars are
per-partition tensor_scalar ops.

Attention per (batch, head): scores psum[q=128, k=256] = qT^T kT (K=DH=64,
row-group packed for even/odd heads); exp on ScalarE with accumulated row sums
(no max-subtraction needed: |scores| <~ 1 by construction); the transpose of P
needed for P@V is a regular matmul with diag(1/rowsum) as the moving operand,
folding the softmax normalization in for free; P^T then feeds
attnT[dh, q] = V-lhsT matmuls (col-group packed head pairs).

Both BN affines are folded into the adjacent matmuls rather than applied as
elementwise passes: a2/c2 go into the QKV weights (scaled in-place on device;
Q-bias corrected by a small W^T c matvec; the K/V corrections are provably
softmax/BN-invariant and dropped) and the residual+BN becomes an extra
diag(a) matmul accumulated into the O-proj / W2 psum with the +c added by the
psum-drain copy. rstd is computed on VectorE only (quake rsqrt + 2 Newton
steps) so ScalarE keeps a single activation table (exp) all run long.

Matmul inputs are bf16 (fp32 PSUM accumulate); the residual stream hT stays
fp32. The embedding gather runs on-device as a one-hot matmul: the host builds
a sparse one-hot (vocab + seq-position rows) and the kernel contracts it with
[embed; pos_encodings]. The final BN2-apply + [feat,tok]->[tok,feat]
transpose is fused into per-tile fp32 matmuls (diag(a2) + rank-1 ones x c2row)
feeding contiguous output DMAs.
"""

import numpy as np
import ml_dtypes

# ---------------------------------------------------------------- constants
B, S, D, L, V, H, FF = 128, 256, 512, 4, 96, 8, 512
DH = D // H
EPS = 1e-3
N_CORES = 8
BL = B // N_CORES          # local batches per core
T = BL * S                 # local tokens per core = 4096
P = 128                    # partitions
NF = D // P                # feature tiles = 4
CH = 512                   # token chunk
NCH = T // CH              # chunks = 8
KV = 3                     # one-hot contraction tiles (384 rows / 128)
NT = B * S                 # global token count for BN stats

_BF16 = ml_dtypes.bfloat16

_cache = {}

# tile-pool buffer counts (tunable)
POOL_CFG = dict(cpool1=2, cpool2=2, ppool=8, spool=8,
                opool=3, fpool=2, psA=3, psS=2, psV=3)


def _build_bass(sim=False, boring_final=False, no_matvec=False, sqrt_rstd=False, fold=True,
                no_collective=False):
    """Build the Bass program. sim=True builds a single-core variant with the
    AllReduce replaced by a local DRAM copy (for TimelineSim cost analysis).
    no_collective=True keeps 8 cores but replaces the AllReduce with a local
    DRAM roundtrip (numerically wrong; for collective-cost measurement)."""
    import concourse.bacc as bacc
    import concourse.tile as tile
    from concourse import mybir
    from concourse.masks import make_identity

    f32 = mybir.dt.float32
    bf16 = mybir.dt.bfloat16
    Alu = mybir.AluOpType
    Act = mybir.ActivationFunctionType

    nc = bacc.Bacc("TRN2", target_bir_lowering=False, debug=False,
                   num_devices=1 if sim else N_CORES)

    # ------------------------------------------------------------ dram I/O
    # All inputs are packed host-side into TWO flat dram tensors (one bf16,
    # one f32): the axon/PJRT execute path has a large per-buffer dispatch
    # overhead (~140us per input), so buffer count dominates input bytes.
    n_oh = P * KV * T
    n_embt = P * KV * D
    n_w = L * P * NF * D
    n_vec = L * P * NF
    bf_total = n_oh + n_embt + 6 * n_w + 2 * 6 * n_vec
    packed_bf = nc.dram_tensor("packed_bf", [bf_total], bf16,
                               kind="ExternalInput").ap()
    off = 0

    def take_bf(n, pattern, **axes):
        nonlocal off
        v = packed_bf[off:off + n].rearrange(pattern, **axes)
        off += n
        return v

    onehot_d = take_bf(n_oh, "(p k t) -> p k t", p=P, k=KV, t=T)
    embt_d = take_bf(n_embt, "(p k d) -> p k d", p=P, k=KV, d=D)
    w_d = {}
    for name in ("wq", "wk", "wv", "wo", "w1", "w2"):
        w_d[name] = take_bf(n_w, "(l p f d) -> l p f d", l=L, p=P, f=NF, d=D)
    # the f32 section rides in the same buffer, reinterpreted via bitcast
    f32_sect = packed_bf[off:off + 2 * 6 * n_vec].bitcast(f32)
    voff = 0
    vec_d = {}
    for name in ("bq", "b1", "g1", "be1", "g2", "be2"):
        vec_d[name] = f32_sect[voff:voff + n_vec].rearrange(
            "(l p f) -> l p f", l=L, p=P, f=NF)
        voff += n_vec
    out_d = nc.dram_tensor("out", [T, D], f32, kind="ExternalOutput").ap()

    with tile.TileContext(nc) as tc:
        from contextlib import ExitStack
        ctx = ExitStack()
        with ctx:
            const = ctx.enter_context(tc.tile_pool(name="const", bufs=1))
            hpool = ctx.enter_context(tc.tile_pool(name="h", bufs=1))
            wpool = ctx.enter_context(tc.tile_pool(name="w", bufs=2))
            wpool1 = ctx.enter_context(tc.tile_pool(name="w1p", bufs=1))
            bpool = ctx.enter_context(tc.tile_pool(name="bias", bufs=2))
            stat = ctx.enter_context(tc.tile_pool(name="stat", bufs=2))
            dramp = ctx.enter_context(tc.tile_pool(name="dramp", bufs=2,
                                                   space="DRAM"))

            hT = hpool.tile([P, NF, T], f32)

            ident_bf = const.tile([P, P], bf16)
            make_identity(nc, ident_bf)
            ident_f32 = const.tile([P, P], f32)
            make_identity(nc, ident_f32)
            eps_sb = const.tile([P, 1], f32)
            nc.vector.memset(eps_sb, EPS)
            ones_f32 = const.tile([1, P], f32)
            nc.vector.memset(ones_f32, 1.0)
            magic_sb = const.tile([P, NF], mybir.dt.uint32)
            nc.vector.memset(magic_sb, 0x5F3759DF)
            one_u32 = const.tile([P, NF], mybir.dt.uint32)
            nc.vector.memset(one_u32, 1)

            # ------------------------------------------------ embedding
            with tc.tile_pool(name="embp", bufs=1) as epool, \
                 tc.tile_pool(name="embps", bufs=4, space="PSUM") as eps_pool:
                oh = epool.tile([P, KV, T], bf16)
                emb = epool.tile([P, KV, D], bf16)
                nc.sync.dma_start(emb[:], embt_d[:])
                # split the big one-hot load so layer-0 matmuls can start
                # as soon as the first token-chunks land
                for t8 in range(NCH):
                    nc.sync.dma_start(oh[:, :, t8 * CH:(t8 + 1) * CH],
                                      onehot_d[:, :, t8 * CH:(t8 + 1) * CH])
                for f in range(NF):
                    for t8 in range(NCH):
                        ps = eps_pool.tile([P, CH], f32, tag="eps")
                        for kc in range(KV):
                            nc.tensor.matmul(
                                ps, lhsT=emb[:, kc, f * P:(f + 1) * P],
                                rhs=oh[:, kc, t8 * CH:(t8 + 1) * CH],
                                start=(kc == 0), stop=(kc == KV - 1))
                        dst = hT[:, f, t8 * CH:(t8 + 1) * CH]
                        if t8 % 2 == 0:
                            nc.vector.tensor_copy(out=dst, in_=ps)
                        else:
                            nc.scalar.copy(out=dst, in_=ps)

            # ------------------------------------------------ layer pools
            cpool1 = ctx.enter_context(tc.tile_pool(name="chunk1", bufs=POOL_CFG["cpool1"]))
            cpool2 = ctx.enter_context(tc.tile_pool(name="chunk2", bufs=POOL_CFG["cpool2"]))
            ppool = ctx.enter_context(tc.tile_pool(name="attn", bufs=POOL_CFG["ppool"]))
            spool = ctx.enter_context(tc.tile_pool(name="small", bufs=POOL_CFG["spool"]))
            opool = ctx.enter_context(tc.tile_pool(name="outp", bufs=POOL_CFG["opool"]))
            fpool = ctx.enter_context(tc.tile_pool(name="fold", bufs=POOL_CFG["fpool"]))
            psA = ctx.enter_context(tc.tile_pool(name="psA", bufs=POOL_CFG["psA"],
                                                 space="PSUM"))
            psS = ctx.enter_context(tc.tile_pool(name="psS", bufs=POOL_CFG["psS"],
                                                 space="PSUM"))
            psV = ctx.enter_context(tc.tile_pool(name="psV", bufs=POOL_CFG["psV"],
                                                 space="PSUM"))

            def bn_allreduce(stats_tile, g_sb, be_sb, tag):
                """stats_tile [P, NF, NCH, 6] -> per-feature affine (a, c):
                bn_out = a * z + c, with global (all-core) stats."""
                mv = stat.tile([P, NF, 2], f32, tag=tag + "mv")
                for f in range(NF):
                    nc.vector.bn_aggr(out=mv[:, f, :], in_=stats_tile[:, f, :, :])
                ss = stat.tile([P, NF, 2], f32, tag=tag + "ss")
                tmp = stat.tile([P, NF], f32, tag=tag + "tmp")
                # local sum = mean * T
                nc.vector.tensor_scalar_mul(ss[:, :, 0], mv[:, :, 0], float(T))
                # local sumsq = (var + mean^2) * T
                nc.vector.tensor_tensor(tmp[:], mv[:, :, 0], mv[:, :, 0],
                                        Alu.mult)
                nc.vector.tensor_tensor(tmp[:], tmp[:], mv[:, :, 1], Alu.add)
                # eps folded in here (pre-barrier, off the critical path):
                # after the allreduce, msq - mu^2 = var + eps directly
                nc.vector.tensor_scalar(out=tmp[:], in0=tmp[:],
                                        scalar1=EPS / N_CORES, scalar2=float(T),
                                        op0=Alu.add, op1=Alu.mult)
                nc.vector.tensor_copy(out=ss[:, :, 1], in_=tmp[:])
                din = dramp.tile([P, NF, 2], f32, tag=tag + "din")
                dout = dramp.tile([P, NF, 2], f32, tag=tag + "dout")
                nc.sync.dma_start(din[:], ss[:])
                if sim or no_collective:
                    nc.sync.dma_start(dout[:], din[:])
                else:
                    nc.gpsimd.collective_compute(
                        "AllReduce", Alu.add,
                        replica_groups=[list(range(N_CORES))],
                        ins=[din.opt()], outs=[dout.opt()])
                gs = stat.tile([P, NF, 2], f32, tag=tag + "gs")
                nc.sync.dma_start(gs[:], dout[:])
                mu = stat.tile([P, NF], f32, tag=tag + "mu")
                var = stat.tile([P, NF], f32, tag=tag + "var")
                nt = float(T if sim else NT)
                nc.vector.tensor_scalar_mul(mu[:], gs[:, :, 0], 1.0 / nt)
                nc.vector.tensor_scalar_mul(var[:], gs[:, :, 1], 1.0 / nt)
                nc.vector.tensor_tensor(tmp[:], mu[:], mu[:], Alu.mult)
                nc.vector.tensor_tensor(var[:], var[:], tmp[:], Alu.subtract)
                if sqrt_rstd:
                    nc.scalar.activation(out=var[:], in_=var[:],
                                         func=Act.Sqrt, bias=eps_sb[:])
                    nc.vector.reciprocal(var[:], var[:])
                else:
                    # rstd = 1/sqrt(var+eps) via quake rsqrt + 2 Newton steps,
                    # entirely on DVE — keeps ScalarE's table set pinned to
                    # exp (no ACT_TABLE_LOAD churn on the BN critical path).
                    # (eps already folded into the all-reduced sumsq)
                    y = stat.tile([P, NF], f32, tag=tag + "y")
                    yi = y.bitcast(mybir.dt.uint32)
                    nc.vector.tensor_tensor(
                        yi[:], var.bitcast(mybir.dt.uint32)[:], one_u32[:],
                        Alu.logical_shift_right)
                    nc.vector.tensor_tensor(yi[:], magic_sb[:], yi[:],
                                            Alu.subtract)
                    t2 = stat.tile([P, NF], f32, tag=tag + "t2")
                    for _ in range(2):
                        nc.vector.tensor_tensor(t2[:], y[:], y[:], Alu.mult)
                        nc.vector.tensor_tensor(t2[:], t2[:], var[:], Alu.mult)
                        nc.vector.tensor_scalar(out=t2[:], in0=t2[:],
                                                scalar1=-0.5, scalar2=1.5,
                                                op0=Alu.mult, op1=Alu.add)
                        nc.vector.tensor_tensor(y[:], y[:], t2[:], Alu.mult)
                    nc.vector.tensor_copy(out=var[:], in_=y[:])
                a = stat.tile([P, NF], f32, tag=tag + "a")
                c = stat.tile([P, NF], f32, tag=tag + "c")
                nc.vector.tensor_tensor(a[:], var[:], g_sb[:], Alu.mult)
                nc.vector.tensor_tensor(c[:], mu[:], a[:], Alu.mult)
                nc.vector.tensor_tensor(c[:], be_sb[:], c[:], Alu.subtract)
                return a, c

            def matvec_w_c(wtile, c_bf, tag):
                """out[:, f] (SBUF psum copy) = sum_kc wtile[:, kc, f*P:]^T
                @ c_bf[:, kc] — the W^T c bias correction for BN folding."""
                mv_ps = psS.tile([P, 2 * S], f32, tag="sps",
                                 name=f"mvps_{tag}")
                if no_matvec:
                    nc.vector.memset(mv_ps[:, :NF], 0.0)
                    return mv_ps
                for f in range(NF):
                    for kc in range(NF):
                        nc.tensor.matmul(
                            mv_ps[:, f:f + 1],
                            lhsT=wtile[:, kc, f * P:(f + 1) * P],
                            rhs=c_bf[:, kc:kc + 1],
                            start=(kc == 0), stop=(kc == NF - 1))
                return mv_ps

            def build_diag(a, tag, dt=None, ident=None):
                """diagA[:, f, :] = diag(a[:, f]), for residual+BN folding
                via PE matmul."""
                dA = fpool.tile([P, NF, P], dt or bf16, tag="diagA",
                                name=f"diagA_{tag}")
                idt = ident or ident_bf
                nc.vector.tensor_tensor(
                    dA[:], idt[:, None, :].to_broadcast((P, NF, P)),
                    a[:, :, None].to_broadcast((P, NF, P)), Alu.mult)
                return dA

            a_pend, c_pend = None, None

            for l in range(L):
                w = {}
                for name in ("wq", "wk", "wv", "wo", "w1", "w2"):
                    pool = wpool if name in ("wq", "wk", "wv") else wpool1
                    w[name] = pool.tile([P, NF, D], bf16, tag=name, name=f"{name}_l{l}")
                    nc.sync.dma_start(w[name][:], w_d[name][l])
                vec = {}
                for name in ("bq", "b1", "g1", "be1", "g2", "be2"):
                    vec[name] = bpool.tile([P, NF], f32, tag=name, name=f"{name}_l{l}")
                    nc.sync.dma_start(vec[name][:], vec_d[name][l])

                # ---- fold previous BN2 (a_pend, c_pend) into QKV weights:
                #   Wx^T(a.z + c) = (a.Wx)^T z + Wx^T c
                # The Wk^T c / Wv^T c terms are dropped: a constant per-row
                # shift of K is softmax-invariant, and a constant shift of V
                # becomes a per-feature constant after attention which the
                # next BN removes. Only Q keeps its correction.
                if a_pend is not None and fold:
                    cbf = stat.tile([P, NF], bf16, tag="cbf2",
                                    name=f"cbf2_{l}")
                    nc.vector.tensor_copy(out=cbf[:], in_=c_pend[:])
                    mv_ps = matvec_w_c(w["wq"], cbf, f"q{l}")
                    bqtot = stat.tile([P, NF], f32, tag="bqtot",
                                      name=f"bqtot_{l}")
                    # bqtot = bq/8 (host pre-scaled) + (Wq^T c)/8
                    nc.vector.tensor_scalar(
                        out=bqtot[:], in0=mv_ps[:, :NF], scalar1=0.125,
                        scalar2=None, op0=Alu.mult)
                    nc.vector.tensor_tensor(bqtot[:], bqtot[:], vec["bq"],
                                            Alu.add)
                    for name in ("wq", "wk", "wv"):
                        for kc in range(NF):
                            nc.vector.tensor_scalar_mul(
                                w[name][:, kc, :], w[name][:, kc, :],
                                a_pend[:, kc:kc + 1])
                    diagA2 = build_diag(a_pend, f"a2_{l}")
                else:
                    bqtot = vec["bq"]
                    diagA2 = None
                c2c = c_pend

                # ======================= phase A: attention =======================
                stats1 = stat.tile([P, NF, NCH, 6], f32, tag="st1")
                for c in range(NCH):
                    tsl = slice(c * CH, (c + 1) * CH)
                    # bf16 input activations (plain cast — BN folded into W)
                    hbf = cpool2.tile([P, NF, CH], bf16, tag="hbf")
                    for f in range(NF):
                        if fold or a_pend is None:
                            nc.gpsimd.tensor_copy(out=hbf[:, f, :],
                                                  in_=hT[:, f, tsl])
                        else:
                            nc.gpsimd.tensor_scalar(
                                out=hbf[:, f, :], in0=hT[:, f, tsl],
                                scalar1=a_pend[:, f:f + 1],
                                scalar2=c_pend[:, f:f + 1],
                                op0=Alu.mult, op1=Alu.add)
                    if not fold and a_pend is not None:
                        for f in range(NF):
                            nc.vector.tensor_scalar(
                                out=hT[:, f, tsl], in0=hT[:, f, tsl],
                                scalar1=a_pend[:, f:f + 1],
                                scalar2=c_pend[:, f:f + 1],
                                op0=Alu.mult, op1=Alu.add)

                    # --- Q, K (transposed), V (token-major) projections
                    qT = cpool1.tile([P, NF, CH], bf16, tag="qT")
                    kT = cpool1.tile([P, NF, CH], bf16, tag="kT")
                    vUa = cpool1.tile([P, 4, H, 2 * DH], bf16, tag="vU")
                    for f in range(NF):
                        ps = psA.tile([P, CH], f32, tag="psA")
                        for kc in range(NF):
                            nc.tensor.matmul(
                                ps, lhsT=w["wq"][:, kc, f * P:(f + 1) * P],
                                rhs=hbf[:, kc, :],
                                start=(kc == 0), stop=(kc == NF - 1))
                        nc.vector.tensor_scalar(
                            out=qT[:, f, :], in0=ps, scalar1=0.125,
                            scalar2=bqtot[:, f:f + 1],
                            op0=Alu.mult, op1=Alu.add)
                        ps = psA.tile([P, CH], f32, tag="psA")
                        for kc in range(NF):
                            nc.tensor.matmul(
                                ps, lhsT=w["wk"][:, kc, f * P:(f + 1) * P],
                                rhs=hbf[:, kc, :],
                                start=(kc == 0), stop=(kc == NF - 1))
                        nc.vector.tensor_copy(out=kT[:, f, :], in_=ps)
                    # V drained token-major, 128 columns per head: V
                    # features at [0:64] and 64 ones at [64:128], so the PV
                    # matmul emits attn_unnorm on psum rows 0-63 AND the
                    # softmax rowsum replicated across rows 64-127 — the
                    # drain is then a partition-shifted divide, no row ops.
                    for ts in range(4):
                        ps = psA.tile([P, CH], f32, tag="psA")
                        for kc in range(NF):
                            nc.tensor.matmul(
                                ps, lhsT=hbf[:, kc, ts * P:(ts + 1) * P],
                                rhs=w["wv"][:, kc, :],
                                start=(kc == 0), stop=(kc == NF - 1))
                        nc.vector.tensor_copy(
                            out=vUa[:, ts, :, DH:2 * DH],
                            in_=ps.rearrange("p (h d) -> p h d", h=H))
                    nc.gpsimd.memset(vUa[:, :, :, 0:DH], 1.0)

                    # --- attention: ST[k,q] = kT^T qT directly (no transpose)
                    attnT = cpool1.tile([P, NF, CH], bf16, tag="attnT")
                    for bb in range(2):
                        boff = bb * S
                        for h in range(H):
                            fi, po = h // 2, (h % 2) * DH
                            expP = ppool.tile([P, 2, S], bf16, tag="P",
                                              name=f"P_{c}_{bb}_{h}")
                            st = psS.tile([P, 2, S], f32, tag="sps")
                            for kt in range(2):
                                nc.tensor.matmul(
                                    st[:, kt, :],
                                    lhsT=kT[po:po + DH, fi,
                                            boff + kt * P:boff + (kt + 1) * P],
                                    rhs=qT[po:po + DH, fi, boff:boff + S],
                                    start=True, stop=True)
                            # P^T = exp(scores^T), both k-tiles in one op
                            # (1/8 folded into qT)
                            nc.scalar.activation(out=expP[:], in_=st[:],
                                                 func=Act.Exp)
                            # av rows 0-63: rowsum replicated (ones
                            # block); rows 64-127: unnormalized attn
                            av = psV.tile([2 * DH, S], f32, tag="avps")
                            for kt in range(2):
                                nc.tensor.matmul(
                                    av, lhsT=vUa[:, 2 * bb + kt, h, :],
                                    rhs=expP[:, kt, :],
                                    start=(kt == 0), stop=(kt == 1))
                            # 1/rowsum written partition-shifted to rows
                            # 64-127 so the normalize-multiply has both
                            # inputs at the same partition offset
                            rrep = spool.tile([2 * DH, S], bf16, tag="rr",
                                              name=f"rr_{c}_{bb}_{h}")
                            with nc.allow_low_precision(
                                    reason="1/rowsum in bf16 is plenty"):
                                nc.vector.reciprocal(rrep[DH:2 * DH, :],
                                                     av[0:DH, :])
                            if h % 2 == 0:
                                nc.vector.tensor_tensor(
                                    attnT[po:po + DH, fi, boff:boff + S],
                                    av[DH:2 * DH, :], rrep[DH:2 * DH, :],
                                    Alu.mult)
                            else:
                                avsb = spool.tile([2 * DH, S], bf16,
                                                  tag="avsb",
                                                  name=f"avsb_{c}_{bb}_{h}")
                                nc.scalar.copy(out=avsb[DH:2 * DH, :],
                                               in_=av[DH:2 * DH, :])
                                nc.gpsimd.tensor_tensor(
                                    attnT[po:po + DH, fi, boff:boff + S],
                                    avsb[DH:2 * DH, :], rrep[DH:2 * DH, :],
                                    Alu.mult)
                    # --- O-projection + residual (+ folded BN2 of prev layer)
                    # psum = Wo^T attnT [+ diag(a2) @ z_prev]; copy adds c2.
                    for f in range(NF):
                        ps = psA.tile([P, CH], f32, tag="psA")
                        for kc in range(NF):
                            nc.tensor.matmul(
                                ps, lhsT=w["wo"][:, kc, f * P:(f + 1) * P],
                                rhs=attnT[:, kc, :],
                                start=(kc == 0), stop=False)
                        if diagA2 is not None:
                            nc.tensor.matmul(
                                ps, lhsT=diagA2[:, f, :],
                                rhs=hbf[:, f, :],
                                start=False, stop=True)
                            nc.scalar.activation(
                                out=hT[:, f, tsl], in_=ps, func=Act.Identity,
                                bias=c2c[:, f:f + 1])
                        else:
                            nc.tensor.matmul(
                                ps, lhsT=ident_bf[:], rhs=hbf[:, f, :],
                                start=False, stop=True)
                            nc.scalar.copy(out=hT[:, f, tsl], in_=ps)
                        nc.vector.bn_stats(out=stats1[:, f, c, :],
                                           in_=hT[:, f, tsl])

                a1, c1 = bn_allreduce(stats1, vec["g1"], vec["be1"], "bn1")

                if fold:
                    # ---- fold BN1 into W1:  relu(W1^T(a1.z+c1) + b1)
                    #    = relu((a1.W1)^T z + (b1 + W1^T c1))
                    c1bf = stat.tile([P, NF], bf16, tag="cbf1",
                                     name=f"cbf1_{l}")
                    nc.vector.tensor_copy(out=c1bf[:], in_=c1[:])
                    mv1_ps = matvec_w_c(w["w1"], c1bf, f"w1_{l}")
                    b1tot = stat.tile([P, NF], f32, tag="b1tot",
                                      name=f"b1tot_{l}")
                    nc.vector.tensor_tensor(b1tot[:], mv1_ps[:, :NF],
                                            vec["b1"], Alu.add)
                    for kc in range(NF):
                        nc.vector.tensor_scalar_mul(
                            w["w1"][:, kc, :], w["w1"][:, kc, :],
                            a1[:, kc:kc + 1])
                    diagA1 = build_diag(a1, f"a1_{l}")
                else:
                    b1tot = vec["b1"]
                    diagA1 = None

                # ======================= phase B: FFN =======================
                stats2 = stat.tile([P, NF, NCH, 6], f32, tag="st2")
                for c in range(NCH):
                    tsl = slice(c * CH, (c + 1) * CH)
                    h1bf = cpool2.tile([P, NF, CH], bf16, tag="hbf",
                                       name=f"h1bf_{l}_{c}")
                    for f in range(NF):
                        if fold:
                            nc.gpsimd.tensor_copy(out=h1bf[:, f, :],
                                                  in_=hT[:, f, tsl])
                        else:
                            nc.gpsimd.tensor_scalar(
                                out=h1bf[:, f, :], in0=hT[:, f, tsl],
                                scalar1=a1[:, f:f + 1],
                                scalar2=c1[:, f:f + 1],
                                op0=Alu.mult, op1=Alu.add)
                    if not fold:
                        for f in range(NF):
                            nc.vector.tensor_scalar(
                                out=hT[:, f, tsl], in0=hT[:, f, tsl],
                                scalar1=a1[:, f:f + 1],
                                scalar2=c1[:, f:f + 1],
                                op0=Alu.mult, op1=Alu.add)
                    ffn = cpool2.tile([P, NF, CH], bf16, tag="ffn")
                    for f in range(NF):
                        ps = psA.tile([P, CH], f32, tag="psA")
                        for kc in range(NF):
                            nc.tensor.matmul(
                                ps, lhsT=w["w1"][:, kc, f * P:(f + 1) * P],
                                rhs=h1bf[:, kc, :],
                                start=(kc == 0), stop=(kc == NF - 1))
                        nc.scalar.activation(out=ffn[:, f, :], in_=ps,
                                             func=Act.Relu,
                                             bias=b1tot[:, f:f + 1])
                    # W2 + folded BN1 residual: psum = W2^T ffn + diag(a1) z1;
                    # copy adds c1 per feature.
                    for f in range(NF):
                        ps = psA.tile([P, CH], f32, tag="psA")
                        for kc in range(NF):
                            nc.tensor.matmul(
                                ps, lhsT=w["w2"][:, kc, f * P:(f + 1) * P],
                                rhs=ffn[:, kc, :],
                                start=(kc == 0), stop=False)
                        if fold:
                            nc.tensor.matmul(
                                ps, lhsT=diagA1[:, f, :], rhs=h1bf[:, f, :],
                                start=False, stop=True)
                            nc.scalar.activation(
                                out=hT[:, f, tsl], in_=ps, func=Act.Identity,
                                bias=c1[:, f:f + 1])
                        else:
                            nc.tensor.matmul(
                                ps, lhsT=ident_bf[:], rhs=h1bf[:, f, :],
                                start=False, stop=True)
                            nc.scalar.copy(out=hT[:, f, tsl], in_=ps)
                        nc.vector.bn_stats(out=stats2[:, f, c, :],
                                           in_=hT[:, f, tsl])

                a_pend, c_pend = bn_allreduce(stats2, vec["g2"], vec["be2"],
                                              "bn2")

            # ========== final: fused BN2-apply + transpose + store ==========
            # out[t, d] = a2[d]*z[d, t] + c2[d], via matmul with diag(a2)
            # plus a rank-1 (ones x c2row) accumulate.
            if boring_final:
                for c in range(NCH):
                    tsl = slice(c * CH, (c + 1) * CH)
                    for f in range(NF):
                        nc.vector.tensor_scalar(
                            out=hT[:, f, tsl], in0=hT[:, f, tsl],
                            scalar1=a_pend[:, f:f + 1],
                            scalar2=c_pend[:, f:f + 1],
                            op0=Alu.mult, op1=Alu.add)
                diagAF = crow = None
            else:
                diagAF = build_diag(a_pend, "final", dt=f32, ident=ident_f32)
                crow_ps = psS.tile([1, 2 * S], f32, tag="sps", name="crow_ps")
                for f in range(NF):
                    nc.tensor.matmul(
                        crow_ps[0:1, f * P:(f + 1) * P],
                        lhsT=c_pend[:, f:f + 1], rhs=ident_f32[:],
                        start=True, stop=True)
                crow = const.tile([1, NF * P], f32)
                nc.vector.tensor_copy(out=crow[:], in_=crow_ps[0:1, :NF * P])
            for tt in range(T // P):
                ops = psA.tile([P, CH], f32, tag="psA")
                for f in range(NF):
                    if boring_final:
                        nc.tensor.matmul(
                            ops[:, f * P:(f + 1) * P],
                            lhsT=hT[:, f, tt * P:(tt + 1) * P],
                            rhs=ident_f32[:],
                            start=True, stop=True)
                        continue
                    nc.tensor.matmul(
                        ops[:, f * P:(f + 1) * P],
                        lhsT=hT[:, f, tt * P:(tt + 1) * P],
                        rhs=diagAF[:, f, :],
                        start=True, stop=False)
                    nc.tensor.matmul(
                        ops[:, f * P:(f + 1) * P],
                        lhsT=ones_f32[:], rhs=crow[0:1, f * P:(f + 1) * P],
                        start=False, stop=True)
                ot = opool.tile([P, CH], f32, tag="ot")
                if tt % 2 == 0:
                    nc.vector.tensor_copy(out=ot[:], in_=ops)
                else:
                    nc.scalar.copy(out=ot[:], in_=ops)
                nc.sync.dma_start(out_d[tt * P:(tt + 1) * P, :], ot[:])

    nc.compile()
    return nc


def _host_prep(inputs):
    """Build per-core in_maps from the full inputs."""
    seq = np.asarray(inputs["sequence"])
    pos = np.asarray(inputs["pos_encodings"], dtype=np.float32)
    emb = np.asarray(inputs["embed"], dtype=np.float32)

    # extended embedding table: rows 0..95 vocab, 96..351 positions, pad to 384
    embt = np.zeros((KV * P, D), np.float32)
    embt[:V] = emb
    embt[V:V + S] = pos
    embt = np.ascontiguousarray(
        embt.reshape(KV, P, D).transpose(1, 0, 2))          # [P, KV, D]

    def wprep(wa):  # [L, D, X] -> [L, P, NF, X] bf16
        wa = np.asarray(wa, dtype=np.float32)
        return np.ascontiguousarray(
            wa.reshape(L, NF, P, wa.shape[-1]).transpose(0, 2, 1, 3)
        ).astype(_BF16)

    def vprep(va):  # [L, D] -> [L, P, NF] f32
        va = np.asarray(va, dtype=np.float32)
        return np.ascontiguousarray(va.reshape(L, NF, P).transpose(0, 2, 1))

    # packed-bf16 layout must match _build_bass: onehot, embt, wq..w2
    shared_bf_tail = np.concatenate(
        [embt.astype(_BF16).reshape(-1)]
        + [wprep(inputs[k]).reshape(-1)
           for k in ("Wq", "Wk", "Wv", "Wo", "W1", "W2")])
    # packed-f32 layout must match _build_bass: bq, b1, g1, be1, g2, be2
    # (scores are scaled by 1/sqrt(DH)=1/8 during the Q copy via
    # activation(scale=0.125), which computes in*scale + bias — so the
    # q bias must be pre-scaled here.)
    packed_f32 = np.concatenate([
        vprep(np.asarray(inputs["bq"], np.float32) * 0.125).reshape(-1),
        vprep(inputs["b1"]).reshape(-1),
        vprep(inputs["gamma1"]).reshape(-1),
        vprep(inputs["beta1"]).reshape(-1),
        vprep(inputs["gamma2"]).reshape(-1),
        vprep(inputs["beta2"]).reshape(-1),
    ])

    in_maps = []
    for core in range(N_CORES):
        sl = seq[core * BL:(core + 1) * BL].reshape(T).astype(np.int64)
        onehot = np.zeros((KV * P, T), np.float32)
        tok = np.arange(T)
        onehot[sl, tok] = 1.0                       # vocab row
        onehot[V + (tok % S), tok] = 1.0            # position row
        onehot = np.ascontiguousarray(
            onehot.reshape(KV, P, T).transpose(1, 0, 2)).astype(_BF16)
        m = {
            "packed_bf": np.concatenate(
                [onehot.reshape(-1), shared_bf_tail,
                 packed_f32.view(_BF16)]),
        }
        in_maps.append(m)
    return in_maps


def _run(inputs, trace=False):
    from concourse import bass_utils
    if "nc" not in _cache:
        _cache["nc"] = _build_bass(boring_final=True)
    nc = _cache["nc"]
    in_maps = _host_prep(inputs)
    res = bass_utils.run_bass_kernel_spmd(
        nc, in_maps, core_ids=list(range(N_CORES)), trace=trace)
    outs = [r["out"].reshape(BL, S, D) for r in res.results]
    full = np.concatenate(outs, axis=0).astype(np.float32)
    return full, res


def kernel(**inputs) -> np.ndarray:
    full, _ = _run(inputs, trace=False)
    return full



# revision 22
# speedup vs baseline: 1.0275x; 1.0275x over previous
"""Trainium2 Bass kernel for a 4-layer dense transformer encoder with BatchNorm.

Model (from reference):
  B=128, S=256, D=512, L=4, V=96, H=8, FF=512, DH=64, eps=1e-3
  x = embed[sequence] + pos
  per layer: MHA -> BN(h+attn) -> FFN(relu) -> BN(h+ffn)   (BN in training mode,
  stats over (batch, seq) per feature)

Sharding: data-parallel over batch across 8 cores (16 batches / core).
BN stats are all-reduced (sum, sumsq per feature = 4KB) across cores.

Device layout: activations are kept feature-major ("transposed"):
  hT[feat, token] with feat on partitions (4 tiles of 128) and 4096 tokens free.
All six projections per layer are then natural PE matmuls
  (lhsT = W[feat_in, feat_out], rhs = hT) and BN per-feature scalars are
per-partition tensor_scalar ops.

Attention per (batch, head), transpose-free: scores are computed directly
k-major as ST[k, q] = kT^T qT (contraction DH=64, row-group packed for
even/odd heads), exp on ScalarE (no max-subtraction needed: |scores| <~ 1
by construction). V is drained token-major with 64 ones-columns per head
([ones | V_h] in the lhsT), so the P@V matmul emits the softmax rowsum
replicated on psum rows 0-63 and unnormalized attn on rows 64-127 in one
pass. A DVE reciprocal (written partition-shifted to rows 64-127, the only
offset pattern the BIR verifier allows: both TT inputs must share one
partition offset, only the output may shift) turns the drain into a single
tensor-tensor multiply per head, alternating DVE and ActE+Pool.

Both BN affines are folded into the adjacent matmuls rather than applied as
elementwise passes: a2/c2 go into the QKV weights (scaled in-place on device;
Q-bias corrected by a small W^T c matvec; the K/V corrections are provably
softmax/BN-invariant and dropped) and the residual+BN becomes an extra
diag(a) matmul accumulated into the O-proj / W2 psum with the +c added by the
psum-drain copy. rstd is computed on VectorE only (quake rsqrt + 2 Newton
steps) so ScalarE keeps a single activation table (exp) all run long.

Matmul inputs are bf16 (fp32 PSUM accumulate); the residual stream hT stays
fp32. The embedding gather runs on-device as a one-hot matmul: the host builds
a sparse one-hot (vocab + seq-position rows) and the kernel contracts it with
[embed; pos_encodings]. The final BN2 is applied in place (fp32 tensor_scalar)
and the [feat,tok]->[tok,feat] transpose is per-tile fp32 identity matmuls
feeding contiguous output DMAs.

All host inputs are packed into a SINGLE flat bf16 dram tensor (f32 vectors
ride along via a size-changing bitcast): the axon/PJRT execute path charges
~140us of dispatch overhead PER input buffer, so buffer count dominates.
"""

import numpy as np
import ml_dtypes

# ---------------------------------------------------------------- constants
B, S, D, L, V, H, FF = 128, 256, 512, 4, 96, 8, 512
DH = D // H
EPS = 1e-3
N_CORES = 8
BL = B // N_CORES          # local batches per core
T = BL * S                 # local tokens per core = 4096
P = 128                    # partitions
NF = D // P                # feature tiles = 4
CH = 512                   # token chunk
NCH = T // CH              # chunks = 8
KV = 3                     # one-hot contraction tiles (384 rows / 128)
NT = B * S                 # global token count for BN stats

_BF16 = ml_dtypes.bfloat16

_cache = {}

# tile-pool buffer counts (tunable)
POOL_CFG = dict(cpool1=2, cpool2=2, ppool=8, spool=8,
                opool=3, fpool=2, psA=3, psS=2, psV=3)


def _build_bass(sim=False, boring_final=False, no_matvec=False, sqrt_rstd=False, fold=True,
                no_collective=False):
    """Build the Bass program. sim=True builds a single-core variant with the
    AllReduce replaced by a local DRAM copy (for TimelineSim cost analysis).
    no_collective=True keeps 8 cores but replaces the AllReduce with a local
    DRAM roundtrip (numerically wrong; for collective-cost measurement)."""
    import concourse.bacc as bacc
    import concourse.tile as tile
    from concourse import mybir
    from concourse.masks import make_identity

    f32 = mybir.dt.float32
    bf16 = mybir.dt.bfloat16
    Alu = mybir.AluOpType
    Act = mybir.ActivationFunctionType

    nc = bacc.Bacc("TRN2", target_bir_lowering=False, debug=False,
                   num_devices=1 if sim else N_CORES)

    # ------------------------------------------------------------ dram I/O
    # All inputs are packed host-side into TWO flat dram tensors (one bf16,
    # one f32): the axon/PJRT execute path has a large per-buffer dispatch
    # overhead (~140us per input), so buffer count dominates input bytes.
    n_oh = P * KV * T
    n_embt = P * KV * D
    n_w = L * P * NF * D
    n_vec = L * P * NF
    bf_total = n_oh + n_embt + 6 * n_w + 2 * 6 * n_vec
    packed_bf = nc.dram_tensor("packed_bf", [bf_total], bf16,
                               kind="ExternalInput").ap()
    off = 0

    def take_bf(n, pattern, **axes):
        nonlocal off
        v = packed_bf[off:off + n].rearrange(pattern, **axes)
        off += n
        return v

    onehot_d = take_bf(n_oh, "(p k t) -> p k t", p=P, k=KV, t=T)
    embt_d = take_bf(n_embt, "(p k d) -> p k d", p=P, k=KV, d=D)
    w_d = {}
    for name in ("wq", "wk", "wv", "wo", "w1", "w2"):
        w_d[name] = take_bf(n_w, "(l p f d) -> l p f d", l=L, p=P, f=NF, d=D)
    # the f32 section rides in the same buffer, reinterpreted via bitcast
    f32_sect = packed_bf[off:off + 2 * 6 * n_vec].bitcast(f32)
    voff = 0
    vec_d = {}
    for name in ("bq", "b1", "g1", "be1", "g2", "be2"):
        vec_d[name] = f32_sect[voff:voff + n_vec].rearrange(
            "(l p f) -> l p f", l=L, p=P, f=NF)
        voff += n_vec
    out_d = nc.dram_tensor("out", [T, D], f32, kind="ExternalOutput").ap()

    with tile.TileContext(nc) as tc:
        from contextlib import ExitStack
        ctx = ExitStack()
        with ctx:
            const = ctx.enter_context(tc.tile_pool(name="const", bufs=1))
            hpool = ctx.enter_context(tc.tile_pool(name="h", bufs=1))
            wpool = ctx.enter_context(tc.tile_pool(name="w", bufs=2))
            wpool1 = ctx.enter_context(tc.tile_pool(name="w1p", bufs=1))
            bpool = ctx.enter_context(tc.tile_pool(name="bias", bufs=2))
            stat = ctx.enter_context(tc.tile_pool(name="stat", bufs=2))
            dramp = ctx.enter_context(tc.tile_pool(name="dramp", bufs=2,
                                                   space="DRAM"))

            hT = hpool.tile([P, NF, T], f32)

            ident_bf = const.tile([P, P], bf16)
            make_identity(nc, ident_bf)
            ident_f32 = const.tile([P, P], f32)
            make_identity(nc, ident_f32)
            eps_sb = const.tile([P, 1], f32)
            nc.vector.memset(eps_sb, EPS)
            ones_f32 = const.tile([1, P], f32)
            nc.vector.memset(ones_f32, 1.0)
            magic_sb = const.tile([P, NF], mybir.dt.uint32)
            nc.vector.memset(magic_sb, 0x5F3759DF)
            one_u32 = const.tile([P, NF], mybir.dt.uint32)
            nc.vector.memset(one_u32, 1)

            # ------------------------------------------------ embedding
            with tc.tile_pool(name="embp", bufs=1) as epool, \
                 tc.tile_pool(name="embps", bufs=4, space="PSUM") as eps_pool:
                oh = epool.tile([P, KV, T], bf16)
                emb = epool.tile([P, KV, D], bf16)
                nc.sync.dma_start(emb[:], embt_d[:])
                # split the big one-hot load so layer-0 matmuls can start
                # as soon as the first token-chunks land
                for t8 in range(NCH):
                    nc.sync.dma_start(oh[:, :, t8 * CH:(t8 + 1) * CH],
                                      onehot_d[:, :, t8 * CH:(t8 + 1) * CH])
                for f in range(NF):
                    for t8 in range(NCH):
                        ps = eps_pool.tile([P, CH], f32, tag="eps")
                        for kc in range(KV):
                            nc.tensor.matmul(
                                ps, lhsT=emb[:, kc, f * P:(f + 1) * P],
                                rhs=oh[:, kc, t8 * CH:(t8 + 1) * CH],
                                start=(kc == 0), stop=(kc == KV - 1))
                        dst = hT[:, f, t8 * CH:(t8 + 1) * CH]
                        if t8 % 2 == 0:
                            nc.vector.tensor_copy(out=dst, in_=ps)
                        else:
                            nc.scalar.copy(out=dst, in_=ps)

            # ------------------------------------------------ layer pools
            cpool1 = ctx.enter_context(tc.tile_pool(name="chunk1", bufs=POOL_CFG["cpool1"]))
            cpool2 = ctx.enter_context(tc.tile_pool(name="chunk2", bufs=POOL_CFG["cpool2"]))
            ppool = ctx.enter_context(tc.tile_pool(name="attn", bufs=POOL_CFG["ppool"]))
            spool = ctx.enter_context(tc.tile_pool(name="small", bufs=POOL_CFG["spool"]))
            opool = ctx.enter_context(tc.tile_pool(name="outp", bufs=POOL_CFG["opool"]))
            fpool = ctx.enter_context(tc.tile_pool(name="fold", bufs=POOL_CFG["fpool"]))
            psA = ctx.enter_context(tc.tile_pool(name="psA", bufs=POOL_CFG["psA"],
                                                 space="PSUM"))
            psS = ctx.enter_context(tc.tile_pool(name="psS", bufs=POOL_CFG["psS"],
                                                 space="PSUM"))
            psV = ctx.enter_context(tc.tile_pool(name="psV", bufs=POOL_CFG["psV"],
                                                 space="PSUM"))

            def bn_allreduce(stats_tile, g_sb, be_sb, tag):
                """stats_tile [P, NF, NCH, 6] -> per-feature affine (a, c):
                bn_out = a * z + c, with global (all-core) stats."""
                mv = stat.tile([P, NF, 2], f32, tag=tag + "mv")
                for f in range(NF):
                    nc.vector.bn_aggr(out=mv[:, f, :], in_=stats_tile[:, f, :, :])
                ss = stat.tile([P, NF, 2], f32, tag=tag + "ss")
                tmp = stat.tile([P, NF], f32, tag=tag + "tmp")
                # local sum = mean * T
                nc.vector.tensor_scalar_mul(ss[:, :, 0], mv[:, :, 0], float(T))
                # local sumsq = (var + mean^2) * T
                nc.vector.tensor_tensor(tmp[:], mv[:, :, 0], mv[:, :, 0],
                                        Alu.mult)
                nc.vector.tensor_tensor(tmp[:], tmp[:], mv[:, :, 1], Alu.add)
                # eps folded in here (pre-barrier, off the critical path):
                # after the allreduce, msq - mu^2 = var + eps directly
                nc.vector.tensor_scalar(out=tmp[:], in0=tmp[:],
                                        scalar1=EPS / N_CORES, scalar2=float(T),
                                        op0=Alu.add, op1=Alu.mult)
                nc.vector.tensor_copy(out=ss[:, :, 1], in_=tmp[:])
                din = dramp.tile([P, NF, 2], f32, tag=tag + "din")
                dout = dramp.tile([P, NF, 2], f32, tag=tag + "dout")
                nc.sync.dma_start(din[:], ss[:])
                if sim or no_collective:
                    nc.sync.dma_start(dout[:], din[:])
                else:
                    nc.gpsimd.collective_compute(
                        "AllReduce", Alu.add,
                        replica_groups=[list(range(N_CORES))],
                        ins=[din.opt()], outs=[dout.opt()])
                gs = stat.tile([P, NF, 2], f32, tag=tag + "gs")
                nc.sync.dma_start(gs[:], dout[:])
                mu = stat.tile([P, NF], f32, tag=tag + "mu")
                var = stat.tile([P, NF], f32, tag=tag + "var")
                nt = float(T if sim else NT)
                nc.vector.tensor_scalar_mul(mu[:], gs[:, :, 0], 1.0 / nt)
                nc.vector.tensor_scalar_mul(var[:], gs[:, :, 1], 1.0 / nt)
                nc.vector.tensor_tensor(tmp[:], mu[:], mu[:], Alu.mult)
                nc.vector.tensor_tensor(var[:], var[:], tmp[:], Alu.subtract)
                if sqrt_rstd:
                    nc.scalar.activation(out=var[:], in_=var[:],
                                         func=Act.Sqrt, bias=eps_sb[:])
                    nc.vector.reciprocal(var[:], var[:])
                else:
                    # rstd = 1/sqrt(var+eps) via quake rsqrt + 2 Newton steps,
                    # entirely on DVE — keeps ScalarE's table set pinned to
                    # exp (no ACT_TABLE_LOAD churn on the BN critical path).
                    # (eps already folded into the all-reduced sumsq)
                    y = stat.tile([P, NF], f32, tag=tag + "y")
                    yi = y.bitcast(mybir.dt.uint32)
                    nc.vector.tensor_tensor(
                        yi[:], var.bitcast(mybir.dt.uint32)[:], one_u32[:],
                        Alu.logical_shift_right)
                    nc.vector.tensor_tensor(yi[:], magic_sb[:], yi[:],
                                            Alu.subtract)
                    t2 = stat.tile([P, NF], f32, tag=tag + "t2")
                    for _ in range(2):
                        nc.vector.tensor_tensor(t2[:], y[:], y[:], Alu.mult)
                        nc.vector.tensor_tensor(t2[:], t2[:], var[:], Alu.mult)
                        nc.vector.tensor_scalar(out=t2[:], in0=t2[:],
                                                scalar1=-0.5, scalar2=1.5,
                                                op0=Alu.mult, op1=Alu.add)
                        nc.vector.tensor_tensor(y[:], y[:], t2[:], Alu.mult)
                    nc.vector.tensor_copy(out=var[:], in_=y[:])
                a = stat.tile([P, NF], f32, tag=tag + "a")
                c = stat.tile([P, NF], f32, tag=tag + "c")
                nc.vector.tensor_tensor(a[:], var[:], g_sb[:], Alu.mult)
                nc.vector.tensor_tensor(c[:], mu[:], a[:], Alu.mult)
                nc.vector.tensor_tensor(c[:], be_sb[:], c[:], Alu.subtract)
                return a, c

            def matvec_w_c(wtile, c_bf, tag):
                """out[:, f] (SBUF psum copy) = sum_kc wtile[:, kc, f*P:]^T
                @ c_bf[:, kc] — the W^T c bias correction for BN folding."""
                mv_ps = psS.tile([P, 2 * S], f32, tag="sps",
                                 name=f"mvps_{tag}")
                if no_matvec:
                    nc.vector.memset(mv_ps[:, :NF], 0.0)
                    return mv_ps
                for f in range(NF):
                    for kc in range(NF):
                        nc.tensor.matmul(
                            mv_ps[:, f:f + 1],
                            lhsT=wtile[:, kc, f * P:(f + 1) * P],
                            rhs=c_bf[:, kc:kc + 1],
                            start=(kc == 0), stop=(kc == NF - 1))
                return mv_ps

            def build_diag(a, tag, dt=None, ident=None):
                """diagA[:, f, :] = diag(a[:, f]), for residual+BN folding
                via PE matmul."""
                dA = fpool.tile([P, NF, P], dt or bf16, tag="diagA",
                                name=f"diagA_{tag}")
                idt = ident or ident_bf
                nc.vector.tensor_tensor(
                    dA[:], idt[:, None, :].to_broadcast((P, NF, P)),
                    a[:, :, None].to_broadcast((P, NF, P)), Alu.mult)
                return dA

            a_pend, c_pend = None, None

            for l in range(L):
                w = {}
                for name in ("wq", "wk", "wv", "wo", "w1", "w2"):
                    pool = wpool if name in ("wq", "wk", "wv") else wpool1
                    w[name] = pool.tile([P, NF, D], bf16, tag=name, name=f"{name}_l{l}")
                    nc.sync.dma_start(w[name][:], w_d[name][l])
                vec = {}
                for name in ("bq", "b1", "g1", "be1", "g2", "be2"):
                    vec[name] = bpool.tile([P, NF], f32, tag=name, name=f"{name}_l{l}")
                    nc.sync.dma_start(vec[name][:], vec_d[name][l])

                # ---- fold previous BN2 (a_pend, c_pend) into QKV weights:
                #   Wx^T(a.z + c) = (a.Wx)^T z + Wx^T c
                # The Wk^T c / Wv^T c terms are dropped: a constant per-row
                # shift of K is softmax-invariant, and a constant shift of V
                # becomes a per-feature constant after attention which the
                # next BN removes. Only Q keeps its correction.
                if a_pend is not None and fold:
                    cbf = stat.tile([P, NF], bf16, tag="cbf2",
                                    name=f"cbf2_{l}")
                    nc.vector.tensor_copy(out=cbf[:], in_=c_pend[:])
                    mv_ps = matvec_w_c(w["wq"], cbf, f"q{l}")
                    bqtot = stat.tile([P, NF], f32, tag="bqtot",
                                      name=f"bqtot_{l}")
                    # bqtot = bq/8 (host pre-scaled) + (Wq^T c)/8
                    nc.vector.tensor_scalar(
                        out=bqtot[:], in0=mv_ps[:, :NF], scalar1=0.125,
                        scalar2=None, op0=Alu.mult)
                    nc.vector.tensor_tensor(bqtot[:], bqtot[:], vec["bq"],
                                            Alu.add)
                    for name in ("wq", "wk", "wv"):
                        for kc in range(NF):
                            nc.vector.tensor_scalar_mul(
                                w[name][:, kc, :], w[name][:, kc, :],
                                a_pend[:, kc:kc + 1])
                    diagA2 = build_diag(a_pend, f"a2_{l}")
                else:
                    bqtot = vec["bq"]
                    diagA2 = None
                c2c = c_pend

                # ======================= phase A: attention =======================
                stats1 = stat.tile([P, NF, NCH, 6], f32, tag="st1")
                for c in range(NCH):
                    tsl = slice(c * CH, (c + 1) * CH)
                    # bf16 input activations (plain cast — BN folded into W)
                    hbf = cpool2.tile([P, NF, CH], bf16, tag="hbf")
                    for f in range(NF):
                        if fold or a_pend is None:
                            nc.gpsimd.tensor_copy(out=hbf[:, f, :],
                                                  in_=hT[:, f, tsl])
                        else:
                            nc.gpsimd.tensor_scalar(
                                out=hbf[:, f, :], in0=hT[:, f, tsl],
                                scalar1=a_pend[:, f:f + 1],
                                scalar2=c_pend[:, f:f + 1],
                                op0=Alu.mult, op1=Alu.add)
                    if not fold and a_pend is not None:
                        for f in range(NF):
                            nc.vector.tensor_scalar(
                                out=hT[:, f, tsl], in0=hT[:, f, tsl],
                                scalar1=a_pend[:, f:f + 1],
                                scalar2=c_pend[:, f:f + 1],
                                op0=Alu.mult, op1=Alu.add)

                    # --- Q, K (transposed), V (token-major) projections
                    qT = cpool1.tile([P, NF, CH], bf16, tag="qT")
                    kT = cpool1.tile([P, NF, CH], bf16, tag="kT")
                    vUa = cpool1.tile([P, 4, H, 2 * DH], bf16, tag="vU")
                    for f in range(NF):
                        ps = psA.tile([P, CH], f32, tag="psA")
                        for kc in range(NF):
                            nc.tensor.matmul(
                                ps, lhsT=w["wq"][:, kc, f * P:(f + 1) * P],
                                rhs=hbf[:, kc, :],
                                start=(kc == 0), stop=(kc == NF - 1))
                        nc.vector.tensor_scalar(
                            out=qT[:, f, :], in0=ps, scalar1=0.125,
                            scalar2=bqtot[:, f:f + 1],
                            op0=Alu.mult, op1=Alu.add)
                        ps = psA.tile([P, CH], f32, tag="psA")
                        for kc in range(NF):
                            nc.tensor.matmul(
                                ps, lhsT=w["wk"][:, kc, f * P:(f + 1) * P],
                                rhs=hbf[:, kc, :],
                                start=(kc == 0), stop=(kc == NF - 1))
                        nc.vector.tensor_copy(out=kT[:, f, :], in_=ps)
                    # V drained token-major, 128 columns per head: V
                    # features at [0:64] and 64 ones at [64:128], so the PV
                    # matmul emits attn_unnorm on psum rows 0-63 AND the
                    # softmax rowsum replicated across rows 64-127 — the
                    # drain is then a partition-shifted divide, no row ops.
                    for ts in range(4):
                        ps = psA.tile([P, CH], f32, tag="psA")
                        for kc in range(NF):
                            nc.tensor.matmul(
                                ps, lhsT=hbf[:, kc, ts * P:(ts + 1) * P],
                                rhs=w["wv"][:, kc, :],
                                start=(kc == 0), stop=(kc == NF - 1))
                        nc.vector.tensor_copy(
                            out=vUa[:, ts, :, DH:2 * DH],
                            in_=ps.rearrange("p (h d) -> p h d", h=H))
                    nc.gpsimd.memset(vUa[:, :, :, 0:DH], 1.0)

                    # --- attention: ST[k,q] = kT^T qT directly (no transpose)
                    attnT = cpool1.tile([P, NF, CH], bf16, tag="attnT")
                    for bb in range(2):
                        boff = bb * S
                        for h in range(H):
                            fi, po = h // 2, (h % 2) * DH
                            expP = ppool.tile([P, 2, S], bf16, tag="P",
                                              name=f"P_{c}_{bb}_{h}")
                            st = psS.tile([P, 2, S], f32, tag="sps")
                            for kt in range(2):
                                nc.tensor.matmul(
                                    st[:, kt, :],
                                    lhsT=kT[po:po + DH, fi,
                                            boff + kt * P:boff + (kt + 1) * P],
                                    rhs=qT[po:po + DH, fi, boff:boff + S],
                                    start=True, stop=True)
                            # P^T = exp(scores^T), both k-tiles in one op
                            # (1/8 folded into qT)
                            nc.scalar.activation(out=expP[:], in_=st[:],
                                                 func=Act.Exp)
                            # av rows 0-63: rowsum replicated (ones
                            # block); rows 64-127: unnormalized attn
                            av = psV.tile([2 * DH, S], f32, tag="avps")
                            for kt in range(2):
                                nc.tensor.matmul(
                                    av, lhsT=vUa[:, 2 * bb + kt, h, :],
                                    rhs=expP[:, kt, :],
                                    start=(kt == 0), stop=(kt == 1))
                            # 1/rowsum written partition-shifted to rows
                            # 64-127 so the normalize-multiply has both
                            # inputs at the same partition offset
                            rrep = spool.tile([2 * DH, S], bf16, tag="rr",
                                              name=f"rr_{c}_{bb}_{h}")
                            with nc.allow_low_precision(
                                    reason="1/rowsum in bf16 is plenty"):
                                nc.vector.reciprocal(rrep[DH:2 * DH, :],
                                                     av[0:DH, :])
                            if h % 2 == 0:
                                nc.vector.tensor_tensor(
                                    attnT[po:po + DH, fi, boff:boff + S],
                                    av[DH:2 * DH, :], rrep[DH:2 * DH, :],
                                    Alu.mult)
                            else:
                                avsb = spool.tile([2 * DH, S], bf16,
                                                  tag="avsb",
                                                  name=f"avsb_{c}_{bb}_{h}")
                                nc.scalar.copy(out=avsb[DH:2 * DH, :],
                                               in_=av[DH:2 * DH, :])
                                nc.gpsimd.tensor_tensor(
                                    attnT[po:po + DH, fi, boff:boff + S],
                                    avsb[DH:2 * DH, :], rrep[DH:2 * DH, :],
                                    Alu.mult)
                    # --- O-projection + residual (+ folded BN2 of prev layer)
                    # psum = Wo^T attnT [+ diag(a2) @ z_prev]; copy adds c2.
                    for f in range(NF):
                        ps = psA.tile([P, CH], f32, tag="psA")
                        for kc in range(NF):
                            nc.tensor.matmul(
                                ps, lhsT=w["wo"][:, kc, f * P:(f + 1) * P],
                                rhs=attnT[:, kc, :],
                                start=(kc == 0), stop=False)
                        if diagA2 is not None:
                            nc.tensor.matmul(
                                ps, lhsT=diagA2[:, f, :],
                                rhs=hbf[:, f, :],
                                start=False, stop=True)
                            nc.scalar.activation(
                                out=hT[:, f, tsl], in_=ps, func=Act.Identity,
                                bias=c2c[:, f:f + 1])
                        else:
                            nc.tensor.matmul(
                                ps, lhsT=ident_bf[:], rhs=hbf[:, f, :],
                                start=False, stop=True)
                            nc.scalar.copy(out=hT[:, f, tsl], in_=ps)
                        nc.vector.bn_stats(out=stats1[:, f, c, :],
                                           in_=hT[:, f, tsl])

                a1, c1 = bn_allreduce(stats1, vec["g1"], vec["be1"], "bn1")

                if fold:
                    # ---- fold BN1 into W1:  relu(W1^T(a1.z+c1) + b1)
                    #    = relu((a1.W1)^T z + (b1 + W1^T c1))
                    c1bf = stat.tile([P, NF], bf16, tag="cbf1",
                                     name=f"cbf1_{l}")
                    nc.vector.tensor_copy(out=c1bf[:], in_=c1[:])
                    mv1_ps = matvec_w_c(w["w1"], c1bf, f"w1_{l}")
                    b1tot = stat.tile([P, NF], f32, tag="b1tot",
                                      name=f"b1tot_{l}")
                    nc.vector.tensor_tensor(b1tot[:], mv1_ps[:, :NF],
                                            vec["b1"], Alu.add)
                    for kc in range(NF):
                        nc.vector.tensor_scalar_mul(
                            w["w1"][:, kc, :], w["w1"][:, kc, :],
                            a1[:, kc:kc + 1])
                    diagA1 = build_diag(a1, f"a1_{l}")
                else:
                    b1tot = vec["b1"]
                    diagA1 = None

                # ======================= phase B: FFN =======================
                stats2 = stat.tile([P, NF, NCH, 6], f32, tag="st2")
                for c in range(NCH):
                    tsl = slice(c * CH, (c + 1) * CH)
                    h1bf = cpool2.tile([P, NF, CH], bf16, tag="hbf",
                                       name=f"h1bf_{l}_{c}")
                    for f in range(NF):
                        if fold:
                            nc.gpsimd.tensor_copy(out=h1bf[:, f, :],
                                                  in_=hT[:, f, tsl])
                        else:
                            nc.gpsimd.tensor_scalar(
                                out=h1bf[:, f, :], in0=hT[:, f, tsl],
                                scalar1=a1[:, f:f + 1],
                                scalar2=c1[:, f:f + 1],
                                op0=Alu.mult, op1=Alu.add)
                    if not fold:
                        for f in range(NF):
                            nc.vector.tensor_scalar(
                                out=hT[:, f, tsl], in0=hT[:, f, tsl],
                                scalar1=a1[:, f:f + 1],
                                scalar2=c1[:, f:f + 1],
                                op0=Alu.mult, op1=Alu.add)
                    ffn = cpool2.tile([P, NF, CH], bf16, tag="ffn")
                    for f in range(NF):
                        ps = psA.tile([P, CH], f32, tag="psA")
                        for kc in range(NF):
                            nc.tensor.matmul(
                                ps, lhsT=w["w1"][:, kc, f * P:(f + 1) * P],
                                rhs=h1bf[:, kc, :],
                                start=(kc == 0), stop=(kc == NF - 1))
                        nc.scalar.activation(out=ffn[:, f, :], in_=ps,
                                             func=Act.Relu,
                                             bias=b1tot[:, f:f + 1])
                    # W2 + folded BN1 residual: psum = W2^T ffn + diag(a1) z1;
                    # copy adds c1 per feature.
                    for f in range(NF):
                        ps = psA.tile([P, CH], f32, tag="psA")
                        for kc in range(NF):
                            nc.tensor.matmul(
                                ps, lhsT=w["w2"][:, kc, f * P:(f + 1) * P],
                                rhs=ffn[:, kc, :],
                                start=(kc == 0), stop=False)
                        if fold:
                            nc.tensor.matmul(
                                ps, lhsT=diagA1[:, f, :], rhs=h1bf[:, f, :],
                                start=False, stop=True)
                            nc.scalar.activation(
                                out=hT[:, f, tsl], in_=ps, func=Act.Identity,
                                bias=c1[:, f:f + 1])
                        else:
                            nc.tensor.matmul(
                                ps, lhsT=ident_bf[:], rhs=h1bf[:, f, :],
                                start=False, stop=True)
                            nc.scalar.copy(out=hT[:, f, tsl], in_=ps)
                        nc.vector.bn_stats(out=stats2[:, f, c, :],
                                           in_=hT[:, f, tsl])

                a_pend, c_pend = bn_allreduce(stats2, vec["g2"], vec["be2"],
                                              "bn2")

            # ========== final: fused BN2-apply + transpose + store ==========
            # out[t, d] = a2[d]*z[d, t] + c2[d], via matmul with diag(a2)
            # plus a rank-1 (ones x c2row) accumulate.
            if boring_final:
                for c in range(NCH):
                    tsl = slice(c * CH, (c + 1) * CH)
                    for f in range(NF):
                        nc.vector.tensor_scalar(
                            out=hT[:, f, tsl], in0=hT[:, f, tsl],
                            scalar1=a_pend[:, f:f + 1],
                            scalar2=c_pend[:, f:f + 1],
                            op0=Alu.mult, op1=Alu.add)
                diagAF = crow = None
            else:
                diagAF = build_diag(a_pend, "final", dt=f32, ident=ident_f32)
                crow_ps = psS.tile([1, 2 * S], f32, tag="sps", name="crow_ps")
                for f in range(NF):
                    nc.tensor.matmul(
                        crow_ps[0:1, f * P:(f + 1) * P],
                        lhsT=c_pend[:, f:f + 1], rhs=ident_f32[:],
                        start=True, stop=True)
                crow = const.tile([1, NF * P], f32)
                nc.vector.tensor_copy(out=crow[:], in_=crow_ps[0:1, :NF * P])
            for tt in range(T // P):
                ops = psA.tile([P, CH], f32, tag="psA")
                for f in range(NF):
                    if boring_final:
                        nc.tensor.matmul(
                            ops[:, f * P:(f + 1) * P],
                            lhsT=hT[:, f, tt * P:(tt + 1) * P],
                            rhs=ident_f32[:],
                            start=True, stop=True)
                        continue
                    nc.tensor.matmul(
                        ops[:, f * P:(f + 1) * P],
                        lhsT=hT[:, f, tt * P:(tt + 1) * P],
                        rhs=diagAF[:, f, :],
                        start=True, stop=False)
                    nc.tensor.matmul(
                        ops[:, f * P:(f + 1) * P],
                        lhsT=ones_f32[:], rhs=crow[0:1, f * P:(f + 1) * P],
                        start=False, stop=True)
                ot = opool.tile([P, CH], f32, tag="ot")
                if tt % 2 == 0:
                    nc.vector.tensor_copy(out=ot[:], in_=ops)
                else:
                    nc.scalar.copy(out=ot[:], in_=ops)
                nc.sync.dma_start(out_d[tt * P:(tt + 1) * P, :], ot[:])

    nc.compile()
    return nc


def _host_prep(inputs):
    """Build per-core in_maps from the full inputs."""
    seq = np.asarray(inputs["sequence"])
    pos = np.asarray(inputs["pos_encodings"], dtype=np.float32)
    emb = np.asarray(inputs["embed"], dtype=np.float32)

    # extended embedding table: rows 0..95 vocab, 96..351 positions, pad to 384
    embt = np.zeros((KV * P, D), np.float32)
    embt[:V] = emb
    embt[V:V + S] = pos
    embt = np.ascontiguousarray(
        embt.reshape(KV, P, D).transpose(1, 0, 2))          # [P, KV, D]

    def wprep(wa):  # [L, D, X] -> [L, P, NF, X] bf16
        wa = np.asarray(wa, dtype=np.float32)
        return np.ascontiguousarray(
            wa.reshape(L, NF, P, wa.shape[-1]).transpose(0, 2, 1, 3)
        ).astype(_BF16)

    def vprep(va):  # [L, D] -> [L, P, NF] f32
        va = np.asarray(va, dtype=np.float32)
        return np.ascontiguousarray(va.reshape(L, NF, P).transpose(0, 2, 1))

    # packed-bf16 layout must match _build_bass: onehot, embt, wq..w2
    shared_bf_tail = np.concatenate(
        [embt.astype(_BF16).reshape(-1)]
        + [wprep(inputs[k]).reshape(-1)
           for k in ("Wq", "Wk", "Wv", "Wo", "W1", "W2")])
    # packed-f32 layout must match _build_bass: bq, b1, g1, be1, g2, be2
    # (scores are scaled by 1/sqrt(DH)=1/8 during the Q copy via
    # activation(scale=0.125), which computes in*scale + bias — so the
    # q bias must be pre-scaled here.)
    packed_f32 = np.concatenate([
        vprep(np.asarray(inputs["bq"], np.float32) * 0.125).reshape(-1),
        vprep(inputs["b1"]).reshape(-1),
        vprep(inputs["gamma1"]).reshape(-1),
        vprep(inputs["beta1"]).reshape(-1),
        vprep(inputs["gamma2"]).reshape(-1),
        vprep(inputs["beta2"]).reshape(-1),
    ])

    in_maps = []
    for core in range(N_CORES):
        sl = seq[core * BL:(core + 1) * BL].reshape(T).astype(np.int64)
        onehot = np.zeros((KV * P, T), np.float32)
        tok = np.arange(T)
        onehot[sl, tok] = 1.0                       # vocab row
        onehot[V + (tok % S), tok] = 1.0            # position row
        onehot = np.ascontiguousarray(
            onehot.reshape(KV, P, T).transpose(1, 0, 2)).astype(_BF16)
        m = {
            "packed_bf": np.concatenate(
                [onehot.reshape(-1), shared_bf_tail,
                 packed_f32.view(_BF16)]),
        }
        in_maps.append(m)
    return in_maps


def _run(inputs, trace=False):
    from concourse import bass_utils
    if "nc" not in _cache:
        _cache["nc"] = _build_bass(boring_final=True)
    nc = _cache["nc"]
    in_maps = _host_prep(inputs)
    res = bass_utils.run_bass_kernel_spmd(
        nc, in_maps, core_ids=list(range(N_CORES)), trace=trace)
    outs = [r["out"].reshape(BL, S, D) for r in res.results]
    full = np.concatenate(outs, axis=0).astype(np.float32)
    return full, res


def kernel(**inputs) -> np.ndarray:
    full, _ = _run(inputs, trace=False)
    return full

